# revision 16
# baseline (speedup 1.0000x reference)
"""CTC batch loss (Keras convention, blank = C-1) on 8 Trainium2 NeuronCores.

Strategy (pure data parallel, 128 examples per core = 128 SBUF partitions):
  * Prob-domain scaled DP, split forward/backward: fwd runs t=0..127 from the
    start, bwd runs t=255..128 from the end (reversed state order so both
    recurrences shift the same direction); likelihood = (A alpha_127) . gamma_128.
    127 rounds instead of 255, with each round's ops covering both halves.
  * Label gather via per-example one-hot matmul on the TensorEngine:
    SWDGE cast-DMA f32->bf16 (DRAM->DRAM), XBAR transpose (t,c)->(c,t) split
    over the SP/ACT queues, matmul E_b^T . X^T -> PSUM [64, 256], escape
    copies on DVE/ACT, batched SWDGE re-layout into batch-partitioned p_store.
  * p values pre-shuffled (on Pool, in round-ranges so the DP can start after
    the first range) into round-major p_odd_cat / pm_cat (mask pre-folded), so
    the steady-state round is 5 TT + 2 TS, all contiguous bf16 (2x/4x DVE
    modes), zero cross-engine dependencies.  Rescale every 8 rounds scales the
    state tiles directly; log corrections collected in a strip, reduced once.
"""

import sys
from contextlib import ExitStack

import numpy as np

for _p in ("/opt/trn_rl_repo",):
    if _p not in sys.path:
        sys.path.insert(0, _p)

import concourse.bass as bass
import concourse.tile as tile
from concourse import mybir
from concourse.bass_utils import run_bass_kernel_spmd

# Problem constants (hardcoded per spec nn_CTC_55808805045003)
B, T, C, L = 1024, 256, 128, 64
NCORES = 8
BL = B // NCORES          # 128 examples per core
S = 2 * L + 1             # 129 extended labels
EPS = 1e-7
NR = T // 2               # 128 DP rounds (round 0 = init)
W = 130                   # p_odd_cat row width: 64 fwd + 2 garbage + 64 bwd
GT = 8                    # examples per XBAR transpose
GR = 16                   # examples per batched SWDGE relayout

f32 = mybir.dt.float32
bf16 = mybir.dt.bfloat16
i32 = mybir.dt.int32

ADD = mybir.AluOpType.add
MULT = mybir.AluOpType.mult
AX_X = mybir.AxisListType.X
AFT = mybir.ActivationFunctionType

RESC = 8                  # rescale period (rounds)


def _body(tc, loss_ap, yp, e_ap, mcat_ap, dumps=None):
    nc = tc.nc
    with ExitStack() as ctx:
        const = ctx.enter_context(tc.tile_pool(name="const", bufs=1))
        dstage = ctx.enter_context(tc.tile_pool(name="dstage", bufs=1, space="DRAM"))
        xtp = ctx.enter_context(tc.tile_pool(name="xt", bufs=3))
        escp = ctx.enter_context(tc.tile_pool(name="esc", bufs=6))
        gps = ctx.enter_context(tc.tile_pool(name="gpsum", bufs=6, space="PSUM"))
        tiny = ctx.enter_context(tc.tile_pool(name="tiny", bufs=1))

        ystage = dstage.tile([BL, T, C], bf16)

        # ---- persistent tiles ----
        E = const.tile([128, BL * 65], bf16)          # one-hot+eps, 64 labels+blank
        mcat = const.tile([128, W], bf16)             # skip masks, cat layout
        p_store = const.tile([128, 65 * T], bf16)     # [b, s*256 + t], s=64 blank
        p_blf = const.tile([128, T], f32)
        POC = const.tile([128, NR * W], bf16)         # p_odd_cat, round-major
        PM = const.tile([128, NR * W], bf16)          # mask * p_odd_cat
        MREP = const.tile([128, NR * W], bf16)        # mcat replicated per round

        # host constants in via SWDGE (spreads across all 16 DMA engines)
        nc.gpsimd.dma_start(E[:], e_ap[:, :])
        nc.gpsimd.dma_start(mcat[:], mcat_ap[:, :])

        # ---- gather phase ----
        # 1) f32 -> bf16 cast inline in the DMA (SWDGE), DRAM -> DRAM
        for g in range(BL // GT):
            nc.gpsimd.dma_start(
                ystage[g * GT : (g + 1) * GT, :, :],
                yp[g * GT : (g + 1) * GT, :, :],
            )

        # m_rep: replicate mcat across all rounds by doubling (DVE, no deps on
        # gather data, runs immediately)
        nc.vector.tensor_copy(MREP[:, 0:W], mcat[:])
        n = W
        while n < NR * W:
            m = min(n, NR * W - n)
            nc.vector.tensor_copy(MREP[:, n : n + m], MREP[:, 0:m])
            n += m

        for g in range(BL // GT):
            b0 = g * GT
            # XBAR transpose: [GT*T, C] -> [C, GT*T]; alternate SP/ACT queues
            xt = xtp.tile([128, GT * T], bf16)
            qeng = nc.sync if (g % 2 == 0) else nc.scalar
            qeng.dma_start_transpose(
                xt[:], ystage[b0 : b0 + GT, :, :].rearrange("b t c -> (b t) c")
            )
            for i in range(GT):
                b = b0 + i
                gp = gps.tile([65, T], f32)
                nc.tensor.matmul(
                    gp[:, :],
                    E[:, b * 65 : (b + 1) * 65],
                    xt[:, i * T : (i + 1) * T],
                    start=True,
                    stop=True,
                )
                esc = escp.tile([65, T], bf16, name="esc")
                if b % 2 == 0:
                    nc.vector.tensor_copy(esc[:], gp[:, :])
                else:
                    nc.scalar.copy(esc[:], gp[:, :])
                # per-example re-layout straight into batch-partitioned p_store
                deng = nc.sync if (b % 2 == 0) else nc.scalar
                deng.dma_start(
                    p_store[b : b + 1, :].rearrange("o (s t) -> o s t", s=65),
                    esc[:],
                )

        # blank probs (p_store row s=64) to f32 for the TS scalar slots
        nc.vector.tensor_copy(p_blf[:], p_store[:, 64 * T : 65 * T])

        # ---- p shuffles into round-major layout ----
        ps_ts = p_store[:].rearrange("p (s t) -> p t s", s=65)
        poc3 = POC[:].rearrange("p (r c) -> p r c", c=W)
        # garbage columns 64, 65 stay zero
        nc.vector.memset(poc3[:, :, 64:66], 0.0)
        NRANGE = 4
        RW = NR // NRANGE

        def shuffle_range(k, eng):
            r0, r1 = k * RW, (k + 1) * RW
            if eng is nc.scalar:
                # fwd: POC[., rho, j] = p_store[., j, t=rho]
                nc.scalar.activation(
                    poc3[:, r0:r1, 0:64], ps_ts[:, r0:r1, 0:64], AFT.Copy
                )
                nc.scalar.activation(
                    poc3[:, r0:r1, 66:W],
                    ps_ts[:, 255 - r0 : 255 - r1 : -1, 63::-1],
                    AFT.Copy,
                )
            else:
                eng.tensor_copy(poc3[:, r0:r1, 0:64], ps_ts[:, r0:r1, 0:64])
                eng.tensor_copy(
                    poc3[:, r0:r1, 66:W],
                    ps_ts[:, 255 - r0 : 255 - r1 : -1, 63::-1],
                )

        def pm_range(k):
            r0, r1 = k * RW, (k + 1) * RW
            nc.vector.tensor_tensor(
                PM[:, r0 * W : r1 * W],
                POC[:, r0 * W : r1 * W],
                MREP[:, r0 * W : r1 * W],
                MULT,
            )

        # range 0 on DVE so the DP can start immediately after
        shuffle_range(0, nc.vector)
        pm_range(0)

        # ---- DP state: ST = [AO (cols 0..130) | AE (cols 131..262)] ----
        # AO: col 0 guard, 1..64 fwd odd j (s=2j+1), 65..66 garbage,
        #     67..130 bwd odd m (nu=2m+1).
        # AE (k at col 131+k): k=0 guard, 1..65 fwd even i (s=2i), 66 garbage,
        #     67..131 bwd even m' (nu=2m').
        ST0 = const.tile([128, 264], bf16)
        ST1 = const.tile([128, 264], bf16)
        STs = [ST0, ST1]
        U1 = const.tile([128, 130], bf16)
        U2 = const.tile([128, 131], bf16)
        Z = const.tile([128, 130], bf16)
        Q = const.tile([128, 130], bf16)
        csstore = const.tile([128, 12], f32)
        nc.vector.memset(ST0[:], 0.0)
        nc.vector.memset(ST1[:], 0.0)

        # init (state at t=0 fwd / t=255 bwd) into ST0
        nc.vector.tensor_copy(ST0[:, 132:133], p_blf[:, 0:1])      # AE[1]
        nc.vector.tensor_copy(ST0[:, 1:2], p_store[:, 0:1])        # AO[1]
        nc.vector.tensor_copy(ST0[:, 198:199], p_blf[:, 255:256])  # AE[67]
        nc.vector.tensor_copy(ST0[:, 67:68], p_store[:, 63 * T + 255 : 63 * T + 256])

        cs = tiny.tile([128, 1], f32)
        r_ap = tiny.tile([128, 1], f32)

        SNAPS = sorted(set(range(12, 121, 12)) | {124, 127})
        k_resc = 0
        for rho in range(1, NR):
            Sp = STs[(rho - 1) % 2]
            Sc = STs[rho % 2]
            base = rho * W
            # Pool: u_even and the skip product (reads prev state only)
            nc.gpsimd.tensor_tensor(U2[:], Sp[:, 132:263], Sp[:, 0:131], ADD)
            nc.gpsimd.tensor_tensor(Q[:], Sp[:, 0:130], PM[:, base : base + W], MULT)
            # DVE: u_odd, z, alpha_odd', fwd-even scale
            nc.vector.tensor_tensor(U1[:], Sp[:, 1:131], Sp[:, 132:262], ADD)
            nc.vector.tensor_tensor(Z[:], U1[:], POC[:, base : base + W], MULT)
            nc.vector.tensor_tensor(Sc[:, 1:131], Z[:], Q[:], ADD)
            nc.vector.tensor_scalar(
                Sc[:, 132:198], U2[:, 0:66], p_blf[:, rho : rho + 1], None, MULT
            )
            # ACT: bwd-even scale
            nc.scalar.activation(
                Sc[:, 198:263], U2[:, 66:131], AFT.Copy,
                scale=p_blf[:, 255 - rho : 256 - rho],
            )
            if rho == 24:
                shuffle_range(1, nc.scalar)
                pm_range(1)
            elif rho == 56:
                shuffle_range(2, nc.scalar)
                pm_range(2)
            elif rho == 88:
                shuffle_range(3, nc.scalar)
                pm_range(3)
            if rho in SNAPS:
                nc.vector.tensor_reduce(cs[:], Sc[:, 0:263], AX_X, ADD)
                nc.vector.reciprocal(r_ap[:], cs[:])
                nc.vector.tensor_scalar(Sc[:, 0:263], Sc[:, 0:263], r_ap[:], None, MULT)
                if k_resc < 10:
                    # pre-scale cs by 2^40 (exact): ACT Ln is only accurate
                    # down to ~1e-16; corrected by a constant at the end
                    nc.vector.tensor_scalar(
                        csstore[:, k_resc : k_resc + 1], cs[:], float(2.0 ** 40),
                        None, MULT,
                    )
                else:
                    nc.vector.tensor_copy(csstore[:, k_resc : k_resc + 1], cs[:])
                k_resc += 1
        assert k_resc == 12

        # ---- endgame: L = (A alpha_127) . gamma_128 ----
        STf = STs[(NR - 1) % 2]
        UF = tiny.tile([128, 64], bf16)
        QF = tiny.tile([128, 64], bf16)
        VF = tiny.tile([128, 64], bf16)
        UE = tiny.tile([128, 65], bf16)
        D = tiny.tile([128, 129], bf16)
        nc.vector.tensor_tensor(UF[:], STf[:, 1:65], STf[:, 132:196], ADD)
        nc.vector.tensor_tensor(QF[:], STf[:, 0:64], mcat[:, 0:64], MULT)
        nc.vector.tensor_tensor(VF[:], UF[:], QF[:], ADD)
        nc.vector.tensor_tensor(UE[:], STf[:, 132:197], STf[:, 0:65], ADD)
        nc.vector.tensor_tensor(D[:, 0:64], VF[:], STf[:, 130:66:-1], MULT)
        nc.vector.tensor_tensor(D[:, 64:129], UE[:], STf[:, 262:197:-1], MULT)
        lik = tiny.tile([128, 1], f32)
        nc.vector.tensor_reduce(lik[:], D[:], AX_X, ADD)
        lik2 = tiny.tile([128, 1], f32)
        nc.vector.tensor_scalar(lik2[:], lik[:], float(2.0 ** 64), None, MULT)
        lnlik = tiny.tile([128, 1], f32)
        nc.scalar.activation(lnlik[:], lik2[:], AFT.Ln)
        strip = tiny.tile([128, 12], f32)
        nc.scalar.activation(strip[:], csstore[:], AFT.Ln)
        ssum = tiny.tile([128, 1], f32)
        nc.vector.tensor_reduce(ssum[:], strip[:], AX_X, ADD)
        # loss = -(ln lik2 - 64 ln 2 + 2 * (sum strip - 10*40 ln 2))
        CADD = float((64 + 2 * 10 * 40) * np.log(2.0))
        t1 = tiny.tile([128, 1], f32)
        nc.vector.tensor_scalar(t1[:], ssum[:], -2.0, CADD, MULT, ADD)
        lout = tiny.tile([128, 1], f32)
        nc.vector.scalar_tensor_tensor(lout[:], lnlik[:], -1.0, t1[:], MULT, ADD)
        nc.sync.dma_start(loss_ap[:, :], lout[:])

        if dumps is not None:
            dbg = ctx.enter_context(tc.tile_pool(name="dbg", bufs=1))
            for key, (src_tile, width) in {
                "ps": (p_store, 65 * T),
                "poc": (POC, NR * W),
                "pm": (PM, NR * W),
                "ao": (STf, 131),
            }.items():
                if key not in dumps:
                    continue
                CHW = 4096
                stg = dbg.tile([128, min(width, CHW)], f32, name="dbgstg")
                off = 0
                while off < width:
                    wdt = min(CHW, width - off)
                    nc.vector.tensor_copy(stg[:, 0:wdt], src_tile[:, off : off + wdt])
                    nc.sync.dma_start(dumps[key][:, off : off + wdt], stg[:, 0:wdt])
                    off += wdt
            if "ae" in dumps:
                stg2 = dbg.tile([128, 132], f32, name="dbgstg2")
                nc.vector.tensor_copy(stg2[:], STf[:, 131:263])
                nc.sync.dma_start(dumps["ae"][:, :], stg2[:])
            if "strip" in dumps:
                nc.sync.dma_start(dumps["strip"][:, 0:12], csstore[:])


def build_nc():
    nc = bass.Bass("TRN2", target_bir_lowering=False, debug=False)
    yp = nc.dram_tensor("y_pred", [BL, T, C], f32, kind="ExternalInput").ap()
    e_in = nc.dram_tensor("e_all", [128, BL * 65], bf16, kind="ExternalInput").ap()
    mc_in = nc.dram_tensor("m_cat", [128, W], bf16, kind="ExternalInput").ap()
    loss = nc.dram_tensor("loss", [BL, 1], f32, kind="ExternalOutput").ap()
    with tile.TileContext(nc) as tc:
        _body(tc, loss, yp, e_in, mc_in)
    return nc


def host_label_consts(y_true):
    """E one-hot (+eps, bf16) and cat-layout skip masks: pure label functions."""
    import ml_dtypes

    lab = np.asarray(y_true).astype(np.int64)  # [B, L]
    outs = []
    ar = np.arange(128)
    for i in range(NCORES):
        lb = lab[i * BL : (i + 1) * BL]  # [128, 64]
        # E[c, b*65 + s] = (c == ext[b, s]) + eps, ext = labels then blank
        ext = np.concatenate([lb, np.full((BL, 1), C - 1, np.int64)], axis=1)
        e = (ar[:, None, None] == ext[None, :, :]).astype(np.float32) + EPS
        e = e.astype(ml_dtypes.bfloat16).reshape(128, BL * 65)
        # mcat[b, idx]: idx 0..63 fwd dest j: (lab[j] != lab[j-1]), j>=1
        #              idx 64..65: 0 (garbage)
        #              idx 66+m bwd dest m: (lab[64-m] != lab[63-m]), m>=1
        mc = np.zeros((BL, W), np.float32)
        mc[:, 1:64] = (lb[:, 1:] != lb[:, :-1]).astype(np.float32)
        dif = (lb[:, 1:] != lb[:, :-1]).astype(np.float32)  # [B, 63] at j=1..63
        # bwd m=1..63: mask = dif at position (63-m) i.e. lab[64-m] vs lab[63-m]
        mc[:, 67:130] = dif[:, ::-1]
        outs.append((e, mc.astype(ml_dtypes.bfloat16)))
    return outs


_CACHE = {}

# --- BIR legalizer -----------------------------------------------------------
# This container's walrus encodes at most ONE sync wait on SP-queue
# instruction classes (PSEUDO_DMA_DIRECT2D / XPOSE / CTRL): "Too many sync
# wait commands". Tile freely emits >=2 waits per instruction. Split the
# extras onto NoOps inserted just before (same engine stream => semantics
# preserved, waits satisfied in order).
_SPLIT_OPS = {"DMACopy", "DmaTransposeAnt", "DMAGatherAnt", "Drain", "NoOp"}


def _legalize_bir(bir_bytes):
    import orjson

    d = orjson.loads(bir_bytes)
    n_new = 0
    for fn in d.get("functions", []):
        for blk in fn.get("blocks", []):
            insts = blk.get("instructions")
            if not insts:
                continue
            out = []
            for ins in insts:
                si = ins.get("sync_info")
                if si:
                    waits = si.get("on_wait") or []
                    if len(waits) > 1:
                        for w in waits[:-1]:
                            n_new += 1
                            out.append(
                                {
                                    "debug": ins.get("debug", 0),
                                    "engine": ins["engine"],
                                    "ins": [],
                                    "outs": [],
                                    "name": f"ZW-{n_new}",
                                    "opcode": "NoOp",
                                    "sync_info": {"on_wait": [w], "on_update": []},
                                }
                            )
                        si["on_wait"] = [waits[-1]]
                out.append(ins)
            blk["instructions"] = out
    return orjson.dumps(d)


def _install_bir_legalizer():
    import concourse.bass2jax as b2j

    if getattr(b2j, "_ctc_legalizer_installed", False):
        return
    orig = b2j.compile_bir_kernel

    def wrapper(bir_json, tmpdir, neff_name="file.neff"):
        bir_json = _legalize_bir(bir_json)
        return orig(bir_json, tmpdir, neff_name=neff_name)

    b2j.compile_bir_kernel = wrapper
    b2j._ctc_legalizer_installed = True


def kernel(y_true, y_pred):
    assert y_pred.shape == (B, T, C) and y_true.shape == (B, L)
    _install_bir_legalizer()
    nc = _CACHE.get("nc")
    if nc is None:
        nc = _CACHE["nc"] = build_nc()
    yp = np.ascontiguousarray(y_pred, dtype=np.float32)
    consts = host_label_consts(y_true)
    in_maps = [
        {
            "y_pred": yp[i * BL : (i + 1) * BL],
            "e_all": consts[i][0],
            "m_cat": consts[i][1],
        }
        for i in range(NCORES)
    ]
    res = run_bass_kernel_spmd(nc, in_maps, list(range(NCORES)))
    out = np.concatenate([res.results[i]["loss"] for i in range(NCORES)], axis=0)
    return out.astype(np.float32)
